# revision 1
# baseline (speedup 1.0000x reference)
"""Trainium2 Bass kernel for nn_DConv (diffusion graph conv, K=2, 2 supports).

Contract: kernel(**inputs) takes FULL unsharded inputs (inputs [B,N,D] f32,
adj_vals [E] f32, rows/cols [E] int, weights [D*M,OUT] f32, biases [1,OUT]
f32) and returns the FULL output [B, N, OUT] f32.

Strategy (data-parallel over batch, per the sharding hint):
 - Each of the 8 cores handles B/8 batches: x layout [N, D*Bl] (col = d*Bl+b).
 - Host builds the two normalized supports (vals1,rows->cols / vals2,cols->rows),
   sorts each edge list by destination into 128-node blocks, pads each block's
   edge segment to a multiple of 128 "slots".
 - Device, per spmm: dma_gather (bf16, 512B rows) fetches x[src] per slot;
   a per-chunk [128,128] selection matrix Sel[e, dst_local] = v_e (built on
   the vector engine as (iota==dst)*v) reduces each chunk into PSUM via
   TensorE: y_block += Sel^T @ Z. Eviction is a plain PSUM->bf16 copy; the
   Chebyshev recurrence (x2 = 2*S*x1 - x0) is folded into the projection
   weights on the host, so the 4 spmms produce raw S-products only:
     A1 = S1 X0, R2 = S1 A1, B1 = S2 A1, R4 = S2 B1
   out = X0(W0-W2) + A1(W1-W4) + R2(2 W2) + B1 W3 + R4(2 W4) + bias.
 - Projection: DMA-transpose loads X_m^T tiles, TensorE contracts against a
   host-built block-diagonal W~ [1280, OUT*Bl].
"""
import os
import sys
import numpy as np
import ml_dtypes

for _p in ('/opt/trn_rl_repo', '/root/.axon_site/_ro/trn_rl_repo'):
    if os.path.isdir(_p) and _p not in sys.path:
        sys.path.append(_p)

import concourse.bass as bass
import concourse.mybir as mybir
import concourse.tile as tile
from concourse import bacc
from concourse.bass_utils import run_bass_kernel_spmd

BF16 = ml_dtypes.bfloat16
P = 128
NCORES = 8


# ---------------------------------------------------------------- host prep

def _build_support(vals, src, dst, n_nodes):
    """Sort edges by dst, pad each 128-node block segment to a multiple of
    128 slots. Returns slot arrays + chunk metadata."""
    nb = n_nodes // P
    order = np.argsort(dst, kind='stable')
    s_src = src[order]
    s_dst = dst[order]
    s_v = vals[order]
    blk = (s_dst // P).astype(np.int64)
    cnt = np.bincount(blk, minlength=nb)

    src_parts, dstl_parts, v_parts = [], [], []
    chunk_block = []
    pos = 0
    for b in range(nb):
        c = int(cnt[b])
        nchunk = max(1, -(-c // P))
        pad = nchunk * P - c
        src_parts.append(s_src[pos:pos + c])
        dstl_parts.append(s_dst[pos:pos + c] - b * P)
        v_parts.append(s_v[pos:pos + c])
        if pad:
            src_parts.append(np.zeros(pad, s_src.dtype))
            dstl_parts.append(np.zeros(pad, s_dst.dtype))
            v_parts.append(np.zeros(pad, np.float32))
        chunk_block += [b] * nchunk
        pos += c

    slot_src = np.concatenate(src_parts).astype(np.int16)
    slot_dstl = np.concatenate(dstl_parts).astype(np.float32)
    slot_v = np.concatenate(v_parts).astype(np.float32)
    n_chunks = len(chunk_block)

    # slot-major [128, n_chunks]: arr[p, c] = val[c*128 + p]
    dst_t = np.ascontiguousarray(slot_dstl.reshape(n_chunks, P).T)
    v_t = np.ascontiguousarray(slot_v.reshape(n_chunks, P).T)

    # wrapped idx layout [128, n_slots/16]: tile[p, j] = idx[j*16 + p%16]
    idx = slot_src.reshape(-1, 16).T  # [16, n_slots/16]
    idx_w = np.ascontiguousarray(np.tile(idx, (8, 1)))

    # chunk -> (block, first, last)
    chunk_block = np.asarray(chunk_block)
    first = np.ones(n_chunks, bool)
    first[1:] = chunk_block[1:] != chunk_block[:-1]
    last = np.ones(n_chunks, bool)
    last[:-1] = chunk_block[:-1] != chunk_block[1:]
    return dict(idx_w=idx_w, dst_t=dst_t, v_t=v_t,
                chunk_block=chunk_block, first=first, last=last,
                n_chunks=n_chunks)


def preprocess(adj_vals, rows, cols, n_nodes):
    drow = np.zeros(n_nodes, np.float32)
    np.add.at(drow, rows, adj_vals)
    dcol = np.zeros(n_nodes, np.float32)
    np.add.at(dcol, cols, adj_vals)
    inv_drow = np.where(drow > 0, 1.0 / drow, 0.0).astype(np.float32)
    inv_dcol = np.where(dcol > 0, 1.0 / dcol, 0.0).astype(np.float32)
    vals1 = (adj_vals * inv_drow[rows]).astype(np.float32)
    vals2 = (adj_vals * inv_dcol[cols]).astype(np.float32)
    s1 = _build_support(vals1, rows, cols, n_nodes)
    s2 = _build_support(vals2, cols, rows, n_nodes)
    return s1, s2


def build_wtilde(weights, d_in, n_mat, out_dim, bl):
    """W~ [5*d_in*bl, out_dim*bl] bf16 with recurrence folded in.
    Row r = m*(d_in*bl) + (d*bl + b); col = o*bl + b."""
    W = weights.reshape(d_in, n_mat, out_dim)
    C = [W[:, 0] - W[:, 2], W[:, 1] - W[:, 4], 2.0 * W[:, 2], W[:, 3], 2.0 * W[:, 4]]
    F = d_in * bl
    Wt = np.zeros((5 * F, out_dim * bl), np.float32)
    for m in range(5):
        for d in range(d_in):
            for b in range(bl):
                Wt[m * F + d * bl + b, b::bl] = C[m][d]
    return Wt.astype(BF16)


# ---------------------------------------------------------------- program

def build_program(n_nodes, feat, out_feat, sup_metas, call_chunks=64, selg=8):
    """Build the per-core Bass program. sup_metas = (s1, s2) chunk metadata
    (only n_chunks/chunk_block/first/last are used — the program layout
    depends on them)."""
    ob = 256  # out_dim * bl
    nt = n_nodes // P  # projection node tiles
    n_wchunks = 5 * feat // P

    nc = bacc.Bacc("TRN2", target_bir_lowering=False, debug=False,
                   num_devices=NCORES)
    dt = mybir.dt

    x0 = nc.dram_tensor("x0", [n_nodes, feat], dt.bfloat16, kind="ExternalInput")
    iota_in = nc.dram_tensor("iota", [P, P], dt.float32, kind="ExternalInput")
    wt_in = nc.dram_tensor("wt", [5 * feat, ob], dt.bfloat16, kind="ExternalInput")
    bias_in = nc.dram_tensor("bias", [P, ob], dt.float32, kind="ExternalInput")

    sup_t = []
    for i, s in enumerate(sup_metas):
        n_slots = s['n_chunks'] * P
        sup_t.append(dict(
            idx=nc.dram_tensor(f"idx{i}", [P, n_slots // 16], dt.int16,
                               kind="ExternalInput"),
            dst=nc.dram_tensor(f"dst{i}", [P, s['n_chunks']], dt.float32,
                               kind="ExternalInput"),
            v=nc.dram_tensor(f"v{i}", [P, s['n_chunks']], dt.float32,
                             kind="ExternalInput"),
        ))

    A1 = nc.dram_tensor("A1", [n_nodes, feat], dt.bfloat16, kind="Internal")
    R2 = nc.dram_tensor("R2", [n_nodes, feat], dt.bfloat16, kind="Internal")
    B1 = nc.dram_tensor("B1", [n_nodes, feat], dt.bfloat16, kind="Internal")
    R4 = nc.dram_tensor("R4", [n_nodes, feat], dt.bfloat16, kind="Internal")
    out = nc.dram_tensor("out", [n_nodes, ob], dt.float32, kind="ExternalOutput")

    with tile.TileContext(nc) as tc:
        with (
            tc.tile_pool(name="const", bufs=1) as cpool,
            tc.tile_pool(name="z", bufs=2) as zpool,
            tc.tile_pool(name="idx", bufs=2) as ipool,
            tc.tile_pool(name="dv", bufs=2) as dvpool,
            tc.tile_pool(name="sel", bufs=2) as selpool,
            tc.tile_pool(name="ev", bufs=4) as evpool,
            tc.tile_pool(name="lhs", bufs=2) as lpool,
            tc.tile_pool(name="po", bufs=2) as opool,
            tc.tile_pool(name="ps", bufs=4, space="PSUM") as pspool,
            tc.tile_pool(name="pso", bufs=2, space="PSUM") as psopool,
        ):
            iota_sb = cpool.tile([P, P], dt.float32)
            nc.sync.dma_start(iota_sb[:], iota_in[:, :])
            wt_sb = cpool.tile([P, n_wchunks, ob], dt.bfloat16)
            nc.sync.dma_start(
                wt_sb[:],
                wt_in[:, :].rearrange("(k p) o -> p k o", p=P))
            bias_sb = cpool.tile([P, ob], dt.float32)
            nc.sync.dma_start(bias_sb[:], bias_in[:, :])

            def emit_spmm(sup, st, xsrc, ydst):
                n_chunks = sup['n_chunks']
                cb = sup['chunk_block']
                first = sup['first']
                last = sup['last']
                ps = None
                for c0 in range(0, n_chunks, call_chunks):
                    ncall = min(call_chunks, n_chunks - c0)
                    nidx = ncall * P
                    idx_t = ipool.tile([P, call_chunks * 8], dt.int16, tag="idx")
                    nc.sync.dma_start(
                        idx_t[:, :ncall * 8],
                        st['idx'][:, c0 * 8:(c0 + ncall) * 8])
                    dst_t = dvpool.tile([P, call_chunks], dt.float32, tag="dst")
                    nc.sync.dma_start(dst_t[:, :ncall],
                                      st['dst'][:, c0:c0 + ncall])
                    v_t = dvpool.tile([P, call_chunks], dt.float32, tag="v")
                    nc.sync.dma_start(v_t[:, :ncall],
                                      st['v'][:, c0:c0 + ncall])
                    z_t = zpool.tile([P, call_chunks, feat], dt.bfloat16, tag="z")
                    nc.gpsimd.dma_gather(
                        z_t[:, :ncall, :], xsrc[:, :], idx_t[:, :ncall * 8],
                        nidx, nidx, feat, single_packet=False)
                    sel_t = selpool.tile([P, call_chunks, P], dt.bfloat16,
                                         tag="sel")
                    for g0 in range(0, ncall, selg):
                        ng = min(selg, ncall - g0)
                        sel_sl = sel_t[:, g0:g0 + ng, :]
                        nc.vector.tensor_tensor(
                            out=sel_sl,
                            in0=iota_sb[:][:, None, :].to_broadcast([P, ng, P]),
                            in1=dst_t[:, g0:g0 + ng, None].to_broadcast([P, ng, P]),
                            op=mybir.AluOpType.is_equal)
                        nc.vector.tensor_tensor(
                            out=sel_sl,
                            in0=sel_sl,
                            in1=v_t[:, g0:g0 + ng, None].to_broadcast([P, ng, P]),
                            op=mybir.AluOpType.mult)
                    for cl in range(ncall):
                        c = c0 + cl
                        if first[c]:
                            ps = pspool.tile([P, feat], dt.float32, tag="ps")
                        nc.tensor.matmul(
                            out=ps[:],
                            lhsT=sel_t[:, cl, :],
                            rhs=z_t[:, cl, :],
                            start=bool(first[c]),
                            stop=bool(last[c]),
                        )
                        if last[c]:
                            b = cb[c]
                            y_sb = evpool.tile([P, feat], dt.bfloat16, tag="y")
                            nc.vector.tensor_copy(out=y_sb[:], in_=ps[:])
                            nc.sync.dma_start(
                                ydst[b * P:(b + 1) * P, :], y_sb[:])

            emit_spmm(sup_metas[0], sup_t[0], x0, A1)
            emit_spmm(sup_metas[0], sup_t[0], A1, R2)
            emit_spmm(sup_metas[1], sup_t[1], A1, B1)
            emit_spmm(sup_metas[1], sup_t[1], B1, R4)

            # projection
            xs = [x0, A1, R2, B1, R4]
            for t in range(nt):
                rows = slice(t * P, (t + 1) * P)
                pso = psopool.tile([P, ob], dt.float32, tag="pso")
                for k in range(n_wchunks):
                    m, h = divmod(k, feat // P)
                    lhsT = lpool.tile([P, P], dt.bfloat16, tag="lhsT")
                    nc.sync.dma_start_transpose(
                        lhsT[:], xs[m][rows, h * P:(h + 1) * P])
                    nc.tensor.matmul(
                        out=pso[:],
                        lhsT=lhsT[:],
                        rhs=wt_sb[:, k, :],
                        start=(k == 0),
                        stop=(k == n_wchunks - 1),
                    )
                o_sb = opool.tile([P, ob], dt.float32, tag="osb")
                nc.vector.tensor_tensor(out=o_sb[:], in0=pso[:],
                                        in1=bias_sb[:],
                                        op=mybir.AluOpType.add)
                nc.sync.dma_start(out[rows, :], o_sb[:])

    nc.compile()
    return nc


# ---------------------------------------------------------------- entry

def _make_core_inputs(core, inputs_f32, s1, s2, wt, bias_rep, n_nodes, d_in):
    bl = inputs_f32.shape[0] // NCORES
    x0 = np.ascontiguousarray(
        inputs_f32[core * bl:(core + 1) * bl]
        .transpose(1, 2, 0).reshape(n_nodes, d_in * bl)).astype(BF16)
    iota = np.tile(np.arange(P, dtype=np.float32)[None, :], (P, 1))
    return dict(
        x0=x0, iota=iota, wt=wt, bias=bias_rep,
        idx0=s1['idx_w'], dst0=s1['dst_t'], v0=s1['v_t'],
        idx1=s2['idx_w'], dst1=s2['dst_t'], v1=s2['v_t'],
    )


def kernel(**inputs):
    inputs_f32 = np.asarray(inputs['inputs'], dtype=np.float32)
    adj_vals = np.asarray(inputs['adj_vals'], dtype=np.float32)
    rows = np.asarray(inputs['rows']).astype(np.int64)
    cols = np.asarray(inputs['cols']).astype(np.int64)
    weights = np.asarray(inputs['weights'], dtype=np.float32)
    biases = np.asarray(inputs['biases'], dtype=np.float32)

    b_total, n_nodes, d_in = inputs_f32.shape
    out_dim = weights.shape[1]
    n_mat = weights.shape[0] // d_in
    bl = b_total // NCORES
    assert n_mat == 5, "kernel is specialized for K=2 (M=5)"

    s1, s2 = preprocess(adj_vals, rows, cols, n_nodes)
    wt = build_wtilde(weights, d_in, n_mat, out_dim, bl)
    bias_rep = np.zeros((P, out_dim * bl), np.float32)
    for o in range(out_dim):
        bias_rep[:, o * bl:(o + 1) * bl] = biases[0, o]

    nc = build_program(n_nodes, d_in * bl, out_dim, (s1, s2))

    in_maps = [
        _make_core_inputs(c, inputs_f32, s1, s2, wt, bias_rep, n_nodes, d_in)
        for c in range(NCORES)
    ]
    res = run_bass_kernel_spmd(nc, in_maps, core_ids=list(range(NCORES)))

    out = np.zeros((b_total, n_nodes, out_dim), np.float32)
    for c in range(NCORES):
        oc = res.results[c]['out']  # [n_nodes, out*bl], col = o*bl + b
        out[c * bl:(c + 1) * bl] = (
            oc.reshape(n_nodes, out_dim, bl).transpose(2, 0, 1))
    return out



# revision 4
# speedup vs baseline: 4.0706x; 4.0706x over previous
"""Trainium2 Bass kernel for nn_DConv (diffusion graph conv, K=2, 2 supports).

Contract: kernel(**inputs) takes FULL unsharded inputs (inputs [B,N,D] f32,
adj_vals [E] f32, rows/cols [E] int, weights [D*M,OUT] f32, biases [1,OUT]
f32) and returns the FULL output [B, N, OUT] f32.

Strategy (1D node partition, per the sharding hint's second clause):
 - Each of the 8 cores owns N/8 = 2048 destination nodes. x tensors are kept
   FULL-WIDTH: [N, B*D] bf16 with col = b*64 + d, so each gathered row is
   4KB. The per-edge dma_gather descriptor-generation cost on GpSimd (Q7)
   is per-INDEX (~8ns), so 8x fewer, 8x fatter gathers per core move the
   spmm from Q7-bound (~50GB/s effective) to DMA-drain-bound (~370GB/s).
 - Per spmm, edges are dst-sorted into the core's 16 node-blocks; each
   block's edge list is padded to a uniform CPB chunks of 128 slots (same
   CPB for every core so all 8 cores run ONE SPMD program; per-core edge
   tables are ExternalInputs). A [128,128] one-hot Sel matrix per chunk
   (built on DVE from iota/is_equal) reduces gathered rows into PSUM via
   TensorE: y_blk += Sel^T @ Z.
 - The Chebyshev recurrence is folded into the projection weights (as in
   the reference-order products): A1 = S1 X0, R2 = S1 A1, B1 = S2 A1,
   R4 = S2 B1; out = X0(W0-W2) + A1(W1-W4) + R2(2W2) + B1 W3 + R4(2W4).
 - Levels 2/3 need FULL A1 / B1 rows as gather sources -> two 8-core
   AllGather collectives (8MB/core). AG(B1) overlaps the R2 spmm.
 - Projection: at eviction each block is transposed on TensorE into
   XT_m [B*D, 2048-own] DRAM tensors; the projection then contracts
   (b,d)-chunks against a block-diagonal [128, 5, 128] weight tile and
   adds bias, writing out [2048, B*OUT] f32 (col = b*64 + o).
"""
import os
import sys
import numpy as np
import ml_dtypes

for _p in ('/opt/trn_rl_repo', '/root/.axon_site/_ro/trn_rl_repo'):
    if os.path.isdir(_p) and _p not in sys.path:
        sys.path.append(_p)

import concourse.bass as bass
import concourse.mybir as mybir
import concourse.tile as tile
from concourse import bacc
from concourse.bass_utils import run_bass_kernel_spmd

BF16 = ml_dtypes.bfloat16
P = 128
NCORES = 8
GCH = 8  # chunks per dma_gather call (8*128 = 1024 indices, 4MB)


# ---------------------------------------------------------------- host prep

def _build_support(vals, src, dst, n_nodes):
    """dst-sort edges, partition into NCORES x 16 blocks of 128 dst nodes,
    pad every block to a uniform CPB*128 slots. Returns per-core tables."""
    nb_total = n_nodes // P          # 128 blocks across all cores
    order = np.argsort(dst, kind='stable')
    s_src = src[order].astype(np.int64)
    s_dst = dst[order].astype(np.int64)
    s_v = vals[order].astype(np.float32)
    blk = s_dst // P
    cnt = np.bincount(blk, minlength=nb_total)
    cpb = int(max(1, -(-cnt.max() // P)))
    spb = cpb * P                     # slots per block

    starts = np.zeros(nb_total, np.int64)
    starts[1:] = np.cumsum(cnt)[:-1]
    pos_in_blk = np.arange(len(s_src)) - starts[blk]
    slot = blk * spb + pos_in_blk

    idx_flat = np.zeros(nb_total * spb, np.int16)
    v_flat = np.zeros(nb_total * spb, np.float32)
    dstl_flat = np.zeros(nb_total * spb, np.float32)
    idx_flat[slot] = s_src.astype(np.int16)
    v_flat[slot] = s_v
    dstl_flat[slot] = (s_dst % P).astype(np.float32)

    blocks_per_core = nb_total // NCORES  # 16
    cores = []
    for c in range(NCORES):
        lo = c * blocks_per_core * spb
        hi = (c + 1) * blocks_per_core * spb
        sl_idx = idx_flat[lo:hi]
        sl_v = v_flat[lo:hi]
        sl_d = dstl_flat[lo:hi]
        n_slots = hi - lo
        n_chunks = n_slots // P
        idx_w = np.ascontiguousarray(
            np.tile(sl_idx.reshape(-1, 16).T, (8, 1)))          # [128, n/16]
        dst_t = np.ascontiguousarray(sl_d.reshape(n_chunks, P).T)  # [128, nch]
        v_t = np.ascontiguousarray(sl_v.reshape(n_chunks, P).T)
        cores.append(dict(idx_w=idx_w, dst_t=dst_t, v_t=v_t))
    return dict(cpb=cpb, n_chunks=blocks_per_core * cpb, cores=cores)


def preprocess(adj_vals, rows, cols, n_nodes):
    drow = np.zeros(n_nodes, np.float32)
    np.add.at(drow, rows, adj_vals)
    dcol = np.zeros(n_nodes, np.float32)
    np.add.at(dcol, cols, adj_vals)
    inv_drow = np.where(drow > 0, 1.0 / drow, 0.0).astype(np.float32)
    inv_dcol = np.where(dcol > 0, 1.0 / dcol, 0.0).astype(np.float32)
    vals1 = (adj_vals * inv_drow[rows]).astype(np.float32)
    vals2 = (adj_vals * inv_dcol[cols]).astype(np.float32)
    s1 = _build_support(vals1, rows, cols, n_nodes)
    s2 = _build_support(vals2, cols, rows, n_nodes)
    return s1, s2


def build_wt(weights, d_in, out_dim):
    """[128 (bj*64+d), 5, 128 (bj*64+o)] bf16, block-diagonal over bj in
    {0,1}, with the Chebyshev recurrence folded in."""
    W = weights.reshape(d_in, 5, out_dim)
    C = [W[:, 0] - W[:, 2], W[:, 1] - W[:, 4], 2.0 * W[:, 2], W[:, 3],
         2.0 * W[:, 4]]
    Wt = np.zeros((P, 5, P), np.float32)
    for m in range(5):
        for bj in range(2):
            Wt[bj * 64:(bj + 1) * 64, m, bj * 64:(bj + 1) * 64] = C[m]
    return np.ascontiguousarray(Wt.reshape(P, 5 * P)).astype(BF16)


# ---------------------------------------------------------------- program

def build_program(n_nodes, feat, cpbs):
    """feat = B*D (2048). cpbs = (cpb_s1, cpb_s2)."""
    NL = n_nodes // NCORES            # 2048 own nodes
    NB = NL // P                      # 16 blocks
    FC = feat // P                    # 16 column chunks

    nc = bacc.Bacc("TRN2", target_bir_lowering=False, debug=False,
                   num_devices=NCORES)
    dt = mybir.dt

    x0 = nc.dram_tensor("x0", [n_nodes, feat], dt.bfloat16,
                        kind="ExternalInput")
    x0s = nc.dram_tensor("x0s", [NL, feat], dt.bfloat16, kind="ExternalInput")
    iota_in = nc.dram_tensor("iota", [P, P], dt.float32, kind="ExternalInput")
    ident_in = nc.dram_tensor("ident", [P, P], dt.bfloat16,
                              kind="ExternalInput")
    wt_in = nc.dram_tensor("wt", [P, 5 * P], dt.bfloat16, kind="ExternalInput")
    bias_in = nc.dram_tensor("bias", [P, feat], dt.float32,
                             kind="ExternalInput")
    sup_in = []
    for s, cpb in enumerate(cpbs):
        nch = NB * cpb
        sup_in.append(dict(
            idx=nc.dram_tensor(f"idx{s}", [P, nch * 8], dt.int16,
                               kind="ExternalInput"),
            dst=nc.dram_tensor(f"dst{s}", [P, nch], dt.float32,
                               kind="ExternalInput"),
            v=nc.dram_tensor(f"v{s}", [P, nch], dt.float32,
                             kind="ExternalInput"),
            nch=nch, cpb=cpb))

    A1s = nc.dram_tensor("A1s", [NL, feat], dt.bfloat16, kind="Internal")
    B1s = nc.dram_tensor("B1s", [NL, feat], dt.bfloat16, kind="Internal")
    A1f = nc.dram_tensor("A1f", [n_nodes, feat], dt.bfloat16, kind="Internal",
                         addr_space="Shared")
    B1f = nc.dram_tensor("B1f", [n_nodes, feat], dt.bfloat16, kind="Internal",
                         addr_space="Shared")
    XT = [nc.dram_tensor(f"XT{m}", [feat, NL], dt.bfloat16, kind="Internal")
          for m in range(5)]
    out = nc.dram_tensor("out", [NL, feat], dt.float32, kind="ExternalOutput")

    with tile.TileContext(nc) as tc:
        with (
            tc.tile_pool(name="const", bufs=1) as cpool,
            tc.tile_pool(name="z", bufs=2) as zpool,
            tc.tile_pool(name="sel", bufs=2) as selpool,
            tc.tile_pool(name="xin", bufs=2) as xinpool,
            tc.tile_pool(name="y", bufs=2) as ypool,
            tc.tile_pool(name="xts", bufs=2) as xtspool,
            tc.tile_pool(name="xl", bufs=2) as xlpool,
            tc.tile_pool(name="o", bufs=2) as opool,
            tc.tile_pool(name="acc", bufs=1, space="PSUM") as pspool,
            tc.tile_pool(name="tp", bufs=4, space="PSUM") as tppool,
        ):
            iota_sb = cpool.tile([P, P], dt.float32)
            nc.sync.dma_start(iota_sb[:], iota_in[:, :])
            ident_sb = cpool.tile([P, P], dt.bfloat16)
            nc.sync.dma_start(ident_sb[:], ident_in[:, :])
            wt_sb = cpool.tile([P, 5, P], dt.bfloat16)
            nc.sync.dma_start(wt_sb[:], wt_in[:, :].rearrange(
                "p (m o) -> p m o", m=5))
            bias_sb = cpool.tile([P, feat], dt.float32)
            nc.sync.dma_start(bias_sb[:], bias_in[:, :])
            sup_sb = []
            for s, si in enumerate(sup_in):
                ix = cpool.tile([P, si['nch'] * 8], dt.int16, tag=f"ix{s}")
                nc.sync.dma_start(ix[:], si['idx'][:, :])
                ds = cpool.tile([P, si['nch']], dt.float32, tag=f"ds{s}")
                nc.sync.dma_start(ds[:], si['dst'][:, :])
                vv = cpool.tile([P, si['nch']], dt.float32, tag=f"vv{s}")
                nc.sync.dma_start(vv[:], si['v'][:, :])
                sup_sb.append(dict(ix=ix, ds=ds, vv=vv))

            def emit_xt(y_sb, xt_t, nb):
                xts = xtspool.tile([P, FC, P], dt.bfloat16, tag="xts")
                for ci in range(FC):
                    tp = tppool.tile([P, P], dt.bfloat16, tag="tp")
                    nc.tensor.transpose(
                        tp[:], y_sb[:, ci * P:(ci + 1) * P], ident_sb[:])
                    nc.scalar.copy(xts[:, ci, :], tp[:])
                nc.sync.dma_start(
                    xt_t[:, nb * P:(nb + 1) * P].rearrange(
                        "(c p) f -> p c f", p=P),
                    xts[:])

            # phase A: transpose own x0 slice into XT0
            for nb in range(NB):
                xin = xinpool.tile([P, feat], dt.bfloat16, tag="xin")
                nc.sync.dma_start(xin[:], x0s[nb * P:(nb + 1) * P, :])
                emit_xt(xin, XT[0], nb)

            def emit_spmm(s, src, slice_out, xt_t):
                cpb = sup_in[s]['cpb']
                nch = sup_in[s]['nch']
                sb = sup_sb[s]
                acc = None
                for c0 in range(0, nch, GCH):
                    g = min(GCH, nch - c0)
                    z = zpool.tile([P, GCH, feat], dt.bfloat16, tag="z")
                    nc.gpsimd.dma_gather(
                        z[:, :g, :], src[:, :], sb['ix'][:, c0 * 8:(c0 + g) * 8],
                        g * P, g * P, feat, single_packet=False)
                    sel = selpool.tile([P, GCH, P], dt.bfloat16, tag="sel")
                    nc.vector.tensor_tensor(
                        out=sel[:, :g, :],
                        in0=iota_sb[:][:, None, :].to_broadcast([P, g, P]),
                        in1=sb['ds'][:, c0:c0 + g, None].to_broadcast([P, g, P]),
                        op=mybir.AluOpType.is_equal)
                    nc.vector.tensor_tensor(
                        out=sel[:, :g, :],
                        in0=sel[:, :g, :],
                        in1=sb['vv'][:, c0:c0 + g, None].to_broadcast([P, g, P]),
                        op=mybir.AluOpType.mult)
                    for j in range(g):
                        c = c0 + j
                        if c % cpb == 0:
                            acc = pspool.tile([P, feat], dt.float32, tag="acc")
                        for fg in range(0, feat, 512):
                            nc.tensor.matmul(
                                out=acc[:, fg:fg + 512],
                                lhsT=sel[:, j, :],
                                rhs=z[:, j, fg:fg + 512],
                                start=(c % cpb == 0),
                                stop=(c % cpb == cpb - 1))
                        if c % cpb == cpb - 1:
                            nb = c // cpb
                            y = ypool.tile([P, feat], dt.bfloat16, tag="y")
                            nc.vector.tensor_copy(out=y[:], in_=acc[:])
                            if slice_out is not None:
                                nc.sync.dma_start(
                                    slice_out[nb * P:(nb + 1) * P, :], y[:])
                            emit_xt(y, xt_t, nb)

            def ag(slice_t, full_t):
                nc.gpsimd.collective_compute(
                    "AllGather", mybir.AluOpType.bypass,
                    replica_groups=[list(range(NCORES))],
                    ins=[slice_t[:, :]], outs=[full_t[:, :]])

            emit_spmm(0, x0, A1s, XT[1])          # A1 = S1 X0
            ag(A1s, A1f)
            emit_spmm(1, A1f, B1s, XT[3])         # B1 = S2 A1
            ag(B1s, B1f)
            emit_spmm(0, A1f, None, XT[2])        # R2 = S1 A1 (overlaps AG B1)
            emit_spmm(1, B1f, None, XT[4])        # R4 = S2 B1

            # projection
            for nb in range(NB):
                xls = []
                for m in range(5):
                    xl = xlpool.tile([P, FC, P], dt.bfloat16, tag=f"xl{m}")
                    nc.sync.dma_start(
                        xl[:],
                        XT[m][:, nb * P:(nb + 1) * P].rearrange(
                            "(c p) f -> p c f", p=P))
                    xls.append(xl)
                acc = pspool.tile([P, feat], dt.float32, tag="acc")
                for ci in range(FC):
                    for m in range(5):
                        nc.tensor.matmul(
                            out=acc[:, ci * P:(ci + 1) * P],
                            lhsT=xls[m][:, ci, :],
                            rhs=wt_sb[:, m, :],
                            start=(m == 0),
                            stop=(m == 4))
                o = opool.tile([P, feat], dt.float32, tag="o")
                nc.vector.tensor_tensor(out=o[:], in0=acc[:], in1=bias_sb[:],
                                        op=mybir.AluOpType.add)
                nc.sync.dma_start(out[nb * P:(nb + 1) * P, :], o[:])

    nc.compile()
    return nc


# ---------------------------------------------------------------- entry

def prepare(inputs_f32, adj_vals, rows, cols, weights, biases):
    b_total, n_nodes, d_in = inputs_f32.shape
    out_dim = weights.shape[1]
    feat = b_total * d_in
    NL = n_nodes // NCORES

    s1, s2 = preprocess(adj_vals, rows, cols, n_nodes)
    # x layout: col = b*64 + d
    x0_full = np.ascontiguousarray(
        inputs_f32.transpose(1, 0, 2).reshape(n_nodes, feat)).astype(BF16)
    wt = build_wt(weights, d_in, out_dim)
    bias_rep = np.zeros((P, feat), np.float32)
    for b in range(b_total):
        bias_rep[:, b * d_in:b * d_in + out_dim] = biases[0][None, :]
    iota = np.tile(np.arange(P, dtype=np.float32)[None, :], (P, 1))
    ident = np.eye(P, dtype=BF16)

    nc = build_program(n_nodes, feat, (s1['cpb'], s2['cpb']))

    in_maps = []
    for c in range(NCORES):
        in_maps.append(dict(
            x0=x0_full, x0s=x0_full[c * NL:(c + 1) * NL],
            iota=iota, ident=ident, wt=wt, bias=bias_rep,
            idx0=s1['cores'][c]['idx_w'], dst0=s1['cores'][c]['dst_t'],
            v0=s1['cores'][c]['v_t'],
            idx1=s2['cores'][c]['idx_w'], dst1=s2['cores'][c]['dst_t'],
            v1=s2['cores'][c]['v_t'],
        ))
    return nc, in_maps


def assemble(res, b_total, n_nodes, out_dim):
    NL = n_nodes // NCORES
    out = np.zeros((b_total, n_nodes, out_dim), np.float32)
    for c in range(NCORES):
        oc = res.results[c]['out']       # [NL, feat] f32, col = b*64 + o
        d_in = oc.shape[1] // b_total
        out[:, c * NL:(c + 1) * NL, :] = (
            oc.reshape(NL, b_total, d_in)[:, :, :out_dim].transpose(1, 0, 2))
    return out


def kernel(**inputs):
    inputs_f32 = np.asarray(inputs['inputs'], dtype=np.float32)
    adj_vals = np.asarray(inputs['adj_vals'], dtype=np.float32)
    rows = np.asarray(inputs['rows']).astype(np.int64)
    cols = np.asarray(inputs['cols']).astype(np.int64)
    weights = np.asarray(inputs['weights'], dtype=np.float32)
    biases = np.asarray(inputs['biases'], dtype=np.float32)

    b_total, n_nodes, _ = inputs_f32.shape
    out_dim = weights.shape[1]

    nc, in_maps = prepare(inputs_f32, adj_vals, rows, cols, weights, biases)
    res = run_bass_kernel_spmd(nc, in_maps, core_ids=list(range(NCORES)))
    return assemble(res, b_total, n_nodes, out_dim)


# revision 5
# speedup vs baseline: 4.7588x; 1.1691x over previous
"""Trainium2 Bass kernel for nn_DConv (diffusion graph conv, K=2, 2 supports).

Contract: kernel(**inputs) takes FULL unsharded inputs (inputs [B,N,D] f32,
adj_vals [E] f32, rows/cols [E] int, weights [D*M,OUT] f32, biases [1,OUT]
f32) and returns the FULL output [B, N, OUT] f32.

Strategy (1D node partition, per the sharding hint's second clause):
 - Each of the 8 cores owns N/8 = 2048 destination nodes. x tensors are kept
   FULL-WIDTH: [N, B*D] bf16 with col = b*64 + d, so each gathered row is
   4KB. The per-edge dma_gather descriptor-generation cost on GpSimd (Q7)
   is per-INDEX (~8ns), so 8x fewer, 8x fatter gathers per core move the
   spmm from Q7-bound (~50GB/s effective) to DMA/TensorE-bound.
 - Per spmm, edges are dst-sorted into the core's 16 node-blocks; each
   block's edge list is padded to a uniform CPB chunks of 128 slots (same
   CPB for every core so all 8 cores run ONE SPMD program; per-core edge
   tables are ExternalInputs). A [128,128] one-hot Sel matrix per chunk
   (built on DVE from iota/is_equal) reduces gathered rows into PSUM via
   TensorE: y_blk += Sel^T @ Z (4 matmuls of 512 free dim = 1 PSUM bank).
 - The Chebyshev recurrence is folded into the projection weights:
   A1 = S1 X0, R2 = S1 A1, B1 = S2 A1, R4 = S2 B1;
   out = X0(W0-W2) + A1(W1-W4) + R2(2W2) + B1 W3 + R4(2W4).
 - Levels 2/3 need FULL A1 / B1 rows as gather sources -> AllGather, split
   into 4 quarter-collectives overlapped with the producing spmm's tail.
   The gathered-full tensors hold a (quarter, rank)-permuted row order;
   the host permutes the gather indices to match.
 - Projection: at eviction each block is transposed on TensorE into
   XT_m [B*D, 2048-own] DRAM tensors; the projection then contracts
   (b,d)-chunks against a block-diagonal [128, 5, 128] weight tile and
   adds bias, writing out [2048, B*OUT] f32 (col = b*64 + o).
"""
import os
import sys
import numpy as np
import ml_dtypes

for _p in ('/opt/trn_rl_repo', '/root/.axon_site/_ro/trn_rl_repo'):
    if os.path.isdir(_p) and _p not in sys.path:
        sys.path.append(_p)

import concourse.bass as bass
import concourse.mybir as mybir
import concourse.tile as tile
from concourse import bacc
from concourse.bass_utils import run_bass_kernel_spmd

BF16 = ml_dtypes.bfloat16
P = 128
NCORES = 8
GCH = 6   # chunks per dma_gather call (6*128 = 768 indices, 3MB)
NQ = 4    # AllGather quarters


def permute_nodes(n):
    """Row index inside the quarter-AllGathered full tensors:
    node n -> q*4096 + r*512 + (n % 512), q = (n % 2048)//512, r = n//2048."""
    n = np.asarray(n, np.int64)
    return ((n % 2048) // 512) * 4096 + (n // 2048) * 512 + (n % 512)


# ---------------------------------------------------------------- host prep

def _build_support(vals, src, dst, n_nodes):
    """dst-sort edges, partition into NCORES x 16 blocks of 128 dst nodes,
    pad every block to a uniform CPB*128 slots. Returns per-core tables
    (gather idx both in original and permuted node order)."""
    nb_total = n_nodes // P          # 128 blocks across all cores
    order = np.argsort(dst, kind='stable')
    s_src = src[order].astype(np.int64)
    s_dst = dst[order].astype(np.int64)
    s_v = vals[order].astype(np.float32)
    blk = s_dst // P
    cnt = np.bincount(blk, minlength=nb_total)
    cpb = int(max(1, -(-cnt.max() // P)))
    spb = cpb * P                     # slots per block

    starts = np.zeros(nb_total, np.int64)
    starts[1:] = np.cumsum(cnt)[:-1]
    pos_in_blk = np.arange(len(s_src)) - starts[blk]
    slot = blk * spb + pos_in_blk

    idx_flat = np.zeros(nb_total * spb, np.int64)
    v_flat = np.zeros(nb_total * spb, np.float32)
    dstl_flat = np.zeros(nb_total * spb, np.float32)
    idx_flat[slot] = s_src
    v_flat[slot] = s_v
    dstl_flat[slot] = (s_dst % P).astype(np.float32)
    idxp_flat = permute_nodes(idx_flat)

    wrap = lambda a: np.ascontiguousarray(
        np.tile(a.astype(np.int16).reshape(-1, 16).T, (8, 1)))

    blocks_per_core = nb_total // NCORES  # 16
    cores = []
    for c in range(NCORES):
        lo = c * blocks_per_core * spb
        hi = (c + 1) * blocks_per_core * spb
        n_chunks = (hi - lo) // P
        cores.append(dict(
            idx_w=wrap(idx_flat[lo:hi]),
            idxp_w=wrap(idxp_flat[lo:hi]),
            dst_t=np.ascontiguousarray(
                dstl_flat[lo:hi].reshape(n_chunks, P).T),
            v_t=np.ascontiguousarray(v_flat[lo:hi].reshape(n_chunks, P).T)))
    return dict(cpb=cpb, n_chunks=blocks_per_core * cpb, cores=cores)


def preprocess(adj_vals, rows, cols, n_nodes):
    drow = np.zeros(n_nodes, np.float32)
    np.add.at(drow, rows, adj_vals)
    dcol = np.zeros(n_nodes, np.float32)
    np.add.at(dcol, cols, adj_vals)
    inv_drow = np.where(drow > 0, 1.0 / drow, 0.0).astype(np.float32)
    inv_dcol = np.where(dcol > 0, 1.0 / dcol, 0.0).astype(np.float32)
    vals1 = (adj_vals * inv_drow[rows]).astype(np.float32)
    vals2 = (adj_vals * inv_dcol[cols]).astype(np.float32)
    s1 = _build_support(vals1, rows, cols, n_nodes)
    s2 = _build_support(vals2, cols, rows, n_nodes)
    return s1, s2


def build_wt(weights, d_in, out_dim):
    """[128 (bj*64+d), 5, 128 (bj*64+o)] bf16, block-diagonal over bj in
    {0,1}, with the Chebyshev recurrence folded in."""
    W = weights.reshape(d_in, 5, out_dim)
    C = [W[:, 0] - W[:, 2], W[:, 1] - W[:, 4], 2.0 * W[:, 2], W[:, 3],
         2.0 * W[:, 4]]
    Wt = np.zeros((P, 5, P), np.float32)
    for m in range(5):
        for bj in range(2):
            Wt[bj * 64:(bj + 1) * 64, m, bj * 64:(bj + 1) * 64] = C[m]
    return np.ascontiguousarray(Wt.reshape(P, 5 * P)).astype(BF16)


# ---------------------------------------------------------------- program

def build_program(n_nodes, feat, cpbs):
    """feat = B*D (2048). cpbs = (cpb_s1, cpb_s2)."""
    NL = n_nodes // NCORES            # 2048 own nodes
    NB = NL // P                      # 16 blocks
    FC = feat // P                    # 16 column chunks
    QR = NL // NQ                     # 512 rows per AG quarter

    nc = bacc.Bacc("TRN2", target_bir_lowering=False, debug=False,
                   num_devices=NCORES)
    dt = mybir.dt

    x0 = nc.dram_tensor("x0", [n_nodes, feat], dt.bfloat16,
                        kind="ExternalInput")
    x0s = nc.dram_tensor("x0s", [NL, feat], dt.bfloat16, kind="ExternalInput")
    iota_in = nc.dram_tensor("iota", [P, P], dt.float32, kind="ExternalInput")
    ident_in = nc.dram_tensor("ident", [P, P], dt.bfloat16,
                              kind="ExternalInput")
    wt_in = nc.dram_tensor("wt", [P, 5 * P], dt.bfloat16, kind="ExternalInput")
    bias_in = nc.dram_tensor("bias", [P, feat], dt.float32,
                             kind="ExternalInput")
    # gather index tables: s1 original (spmm1), s1 permuted (R2),
    # s2 permuted (B1, R4)
    idx_names = ["idx0", "idx0p", "idx1p"]
    sup_of_idx = [0, 0, 1]
    idx_in = []
    for nm, s in zip(idx_names, sup_of_idx):
        nch = NB * cpbs[s]
        idx_in.append(nc.dram_tensor(nm, [P, nch * 8], dt.int16,
                                     kind="ExternalInput"))
    dv_in = []
    for s, cpb in enumerate(cpbs):
        nch = NB * cpb
        dv_in.append(dict(
            dst=nc.dram_tensor(f"dst{s}", [P, nch], dt.float32,
                               kind="ExternalInput"),
            v=nc.dram_tensor(f"v{s}", [P, nch], dt.float32,
                             kind="ExternalInput"),
            nch=nch, cpb=cpb))

    A1q = [nc.dram_tensor(f"A1q{q}", [QR, feat], dt.bfloat16, kind="Internal")
           for q in range(NQ)]
    B1q = [nc.dram_tensor(f"B1q{q}", [QR, feat], dt.bfloat16, kind="Internal")
           for q in range(NQ)]
    A1f = nc.dram_tensor("A1f", [n_nodes, feat], dt.bfloat16, kind="Internal",
                         addr_space="Shared")
    B1f = nc.dram_tensor("B1f", [n_nodes, feat], dt.bfloat16, kind="Internal",
                         addr_space="Shared")
    XT = [nc.dram_tensor(f"XT{m}", [feat, NL], dt.bfloat16, kind="Internal")
          for m in range(5)]
    out = nc.dram_tensor("out", [NL, feat], dt.float32, kind="ExternalOutput")

    with tile.TileContext(nc) as tc:
        with (
            tc.tile_pool(name="const", bufs=1) as cpool,
            tc.tile_pool(name="z", bufs=3) as zpool,
            tc.tile_pool(name="sel", bufs=2) as selpool,
            tc.tile_pool(name="y", bufs=2) as ypool,
            tc.tile_pool(name="xts", bufs=2) as xtspool,
            tc.tile_pool(name="xl", bufs=2) as xlpool,
            tc.tile_pool(name="o", bufs=2) as opool,
            tc.tile_pool(name="acc", bufs=1, space="PSUM") as pspool,
            tc.tile_pool(name="tp", bufs=4, space="PSUM") as tppool,
        ):
            iota_sb = cpool.tile([P, P], dt.float32)
            nc.sync.dma_start(iota_sb[:], iota_in[:, :])
            ident_sb = cpool.tile([P, P], dt.bfloat16)
            nc.sync.dma_start(ident_sb[:], ident_in[:, :])
            wt_sb = cpool.tile([P, 5, P], dt.bfloat16)
            nc.sync.dma_start(wt_sb[:], wt_in[:, :].rearrange(
                "p (m o) -> p m o", m=5))
            bias_sb = cpool.tile([P, feat], dt.float32)
            nc.sync.dma_start(bias_sb[:], bias_in[:, :])
            ix_sb = []
            for i, (nm, s) in enumerate(zip(idx_names, sup_of_idx)):
                nch = NB * cpbs[s]
                ix = cpool.tile([P, nch * 8], dt.int16, tag=nm)
                nc.sync.dma_start(ix[:], idx_in[i][:, :])
                ix_sb.append(ix)
            dv_sb = []
            for s, si in enumerate(dv_in):
                ds = cpool.tile([P, si['nch']], dt.float32, tag=f"ds{s}")
                nc.sync.dma_start(ds[:], si['dst'][:, :])
                vv = cpool.tile([P, si['nch']], dt.float32, tag=f"vv{s}")
                nc.sync.dma_start(vv[:], si['v'][:, :])
                dv_sb.append(dict(ds=ds, vv=vv))

            def emit_xt(y_sb, xt_t, nb):
                xts = xtspool.tile([P, FC, P], dt.bfloat16, tag="xts")
                for ci in range(FC):
                    tp = tppool.tile([P, P], dt.bfloat16, tag="tp")
                    nc.tensor.transpose(
                        tp[:], y_sb[:, ci * P:(ci + 1) * P], ident_sb[:])
                    nc.scalar.copy(xts[:, ci, :], tp[:])
                nc.sync.dma_start(
                    xt_t[:, nb * P:(nb + 1) * P].rearrange(
                        "(c p) f -> p c f", p=P),
                    xts[:])

            # phase A: transpose own x0 slice into XT0
            for nb in range(NB):
                xin = ypool.tile([P, feat], dt.bfloat16, tag="y")
                nc.sync.dma_start(xin[:], x0s[nb * P:(nb + 1) * P, :])
                emit_xt(xin, XT[0], nb)

            def emit_spmm(s, ix, src, quarters, full_t, xt_t):
                """quarters: list of NQ slice tensors to evict into (or
                None); full_t: AG output or None."""
                cpb = dv_in[s]['cpb']
                nch = dv_in[s]['nch']
                dv = dv_sb[s]
                acc = None
                for c0 in range(0, nch, GCH):
                    g = min(GCH, nch - c0)
                    z = zpool.tile([P, GCH, feat], dt.bfloat16, tag="z")
                    nc.gpsimd.dma_gather(
                        z[:, :g, :], src[:, :], ix[:, c0 * 8:(c0 + g) * 8],
                        g * P, g * P, feat, single_packet=False)
                    sel = selpool.tile([P, GCH, P], dt.bfloat16, tag="sel")
                    nc.vector.tensor_tensor(
                        out=sel[:, :g, :],
                        in0=iota_sb[:][:, None, :].to_broadcast([P, g, P]),
                        in1=dv['ds'][:, c0:c0 + g, None].to_broadcast([P, g, P]),
                        op=mybir.AluOpType.is_equal)
                    nc.vector.tensor_tensor(
                        out=sel[:, :g, :],
                        in0=sel[:, :g, :],
                        in1=dv['vv'][:, c0:c0 + g, None].to_broadcast([P, g, P]),
                        op=mybir.AluOpType.mult)
                    for j in range(g):
                        c = c0 + j
                        if c % cpb == 0:
                            acc = pspool.tile([P, feat], dt.float32, tag="acc")
                        for fg in range(0, feat, 512):
                            nc.tensor.matmul(
                                out=acc[:, fg:fg + 512],
                                lhsT=sel[:, j, :],
                                rhs=z[:, j, fg:fg + 512],
                                start=(c % cpb == 0),
                                stop=(c % cpb == cpb - 1))
                        if c % cpb == cpb - 1:
                            nb = c // cpb
                            y = ypool.tile([P, feat], dt.bfloat16, tag="y")
                            nc.vector.tensor_copy(out=y[:], in_=acc[:])
                            if quarters is not None:
                                q, qb = divmod(nb, NB // NQ)
                                nc.sync.dma_start(
                                    quarters[q][qb * P:(qb + 1) * P, :], y[:])
                                if qb == NB // NQ - 1:
                                    nc.gpsimd.collective_compute(
                                        "AllGather", mybir.AluOpType.bypass,
                                        replica_groups=[list(range(NCORES))],
                                        ins=[quarters[q][:, :]],
                                        outs=[full_t[
                                            q * NCORES * QR:(q + 1) * NCORES * QR,
                                            :]])
                            emit_xt(y, xt_t, nb)

            emit_spmm(0, ix_sb[0], x0, A1q, A1f, XT[1])    # A1 = S1 X0
            emit_spmm(1, ix_sb[2], A1f, B1q, B1f, XT[3])   # B1 = S2 A1
            emit_spmm(0, ix_sb[1], A1f, None, None, XT[2])  # R2 = S1 A1
            emit_spmm(1, ix_sb[2], B1f, None, None, XT[4])  # R4 = S2 B1

            # projection
            for nb in range(NB):
                xls = []
                for m in range(5):
                    xl = xlpool.tile([P, FC, P], dt.bfloat16, tag=f"xl{m}")
                    nc.sync.dma_start(
                        xl[:],
                        XT[m][:, nb * P:(nb + 1) * P].rearrange(
                            "(c p) f -> p c f", p=P))
                    xls.append(xl)
                acc = pspool.tile([P, feat], dt.float32, tag="acc")
                for ci in range(FC):
                    for m in range(5):
                        nc.tensor.matmul(
                            out=acc[:, ci * P:(ci + 1) * P],
                            lhsT=xls[m][:, ci, :],
                            rhs=wt_sb[:, m, :],
                            start=(m == 0),
                            stop=(m == 4))
                o = opool.tile([P, feat], dt.float32, tag="o")
                nc.vector.tensor_tensor(out=o[:], in0=acc[:], in1=bias_sb[:],
                                        op=mybir.AluOpType.add)
                nc.sync.dma_start(out[nb * P:(nb + 1) * P, :], o[:])

    nc.compile()
    return nc


# ---------------------------------------------------------------- entry

def prepare(inputs_f32, adj_vals, rows, cols, weights, biases):
    b_total, n_nodes, d_in = inputs_f32.shape
    out_dim = weights.shape[1]
    feat = b_total * d_in
    NL = n_nodes // NCORES

    s1, s2 = preprocess(adj_vals, rows, cols, n_nodes)
    # x layout: col = b*64 + d
    x0_full = np.ascontiguousarray(
        inputs_f32.transpose(1, 0, 2).reshape(n_nodes, feat)).astype(BF16)
    wt = build_wt(weights, d_in, out_dim)
    bias_rep = np.zeros((P, feat), np.float32)
    for b in range(b_total):
        bias_rep[:, b * d_in:b * d_in + out_dim] = biases[0][None, :]
    iota = np.tile(np.arange(P, dtype=np.float32)[None, :], (P, 1))
    ident = np.eye(P, dtype=BF16)

    nc = build_program(n_nodes, feat, (s1['cpb'], s2['cpb']))

    in_maps = []
    for c in range(NCORES):
        in_maps.append(dict(
            x0=x0_full, x0s=x0_full[c * NL:(c + 1) * NL],
            iota=iota, ident=ident, wt=wt, bias=bias_rep,
            idx0=s1['cores'][c]['idx_w'],
            idx0p=s1['cores'][c]['idxp_w'],
            idx1p=s2['cores'][c]['idxp_w'],
            dst0=s1['cores'][c]['dst_t'], v0=s1['cores'][c]['v_t'],
            dst1=s2['cores'][c]['dst_t'], v1=s2['cores'][c]['v_t'],
        ))
    return nc, in_maps


def assemble(res, b_total, n_nodes, out_dim):
    NL = n_nodes // NCORES
    out = np.zeros((b_total, n_nodes, out_dim), np.float32)
    for c in range(NCORES):
        oc = res.results[c]['out']       # [NL, feat] f32, col = b*64 + o
        d_in = oc.shape[1] // b_total
        out[:, c * NL:(c + 1) * NL, :] = (
            oc.reshape(NL, b_total, d_in)[:, :, :out_dim].transpose(1, 0, 2))
    return out


def kernel(**inputs):
    inputs_f32 = np.asarray(inputs['inputs'], dtype=np.float32)
    adj_vals = np.asarray(inputs['adj_vals'], dtype=np.float32)
    rows = np.asarray(inputs['rows']).astype(np.int64)
    cols = np.asarray(inputs['cols']).astype(np.int64)
    weights = np.asarray(inputs['weights'], dtype=np.float32)
    biases = np.asarray(inputs['biases'], dtype=np.float32)

    b_total, n_nodes, _ = inputs_f32.shape
    out_dim = weights.shape[1]

    nc, in_maps = prepare(inputs_f32, adj_vals, rows, cols, weights, biases)
    res = run_bass_kernel_spmd(nc, in_maps, core_ids=list(range(NCORES)))
    return assemble(res, b_total, n_nodes, out_dim)
